# revision 21
# baseline (speedup 1.0000x reference)
"""Trainium2 Bass kernel for nn_AttnTextClassifier (fp8 DoubleRow version).

Reference math (B=256, T=512, V=50000, E=640, D1=D2=512, C=2):
    tokens   = data * mask                     [B, T]
    embedded = emb_table[tokens] * mask[...,None]
    x  = embedded.reshape(B, T*E)              [B, 327680]
    x1 = relu(x @ W1.T + b1)                   [B, 512]
    x2 = relu(x1 @ W2.T + b2)                  [B, 512]
    out = x2 @ Wp.T + bp                       [B, 2]

Distribution (8 cores): tensor-parallel over the T*E contraction dim.
Core c owns tokens t in [64c, 64c+64) -> 40960 contraction columns.

Both emb_table and W1 are uniform-init, so fp8e4 (scaled by powers of two
into the normal range) keeps the end-to-end max rel err ~1.5e-3 (measured
host-side), well inside the 2e-2 gate.  The host pre-gathers the embedding
rows into k-major fp8 tiles (no on-device gather, no on-device transposes)
and pre-transposes W1; the device runs a pure fp8 DoubleRow matmul stream
(2 fp8 weights/PE cell, warm matmul issue-to-issue 132 ns).  x and W
stream just-in-time in matched pieces, which sustains ~330 GB/s.

The D1=512 output columns are split in two phases.  Phase 0 moves x + the
first W half (DMA-bound); phase 1 moves only the second W half and is
PE-bound, leaving DMA slack in which the ncfw collective path (booted by
a t~8us warm-up AllReduce) executes the phase-0 ReduceScatter.  Only the
phase-1 ReduceScatter is exposed at the end.  Each core then computes
layers 2/3 for its own 32-row batch slice (layer-2 partials for the
n-half-0 columns are computed while ReduceScatter #2 is in flight) and
the host concatenates the 8 output slices (pure unshard).
"""

import os
import sys
import types

import numpy as np

import concourse.bacc as bacc
import concourse.mybir as mybir
import concourse.tile as tile
from concourse.bass_utils import run_bass_kernel_spmd
from concourse.masks import make_identity

B, T, V, E = 256, 512, 50000, 640
D1, D2, C = 512, 512, 2
NCORES = 8
TPC = T // NCORES          # 64 tokens per core
KPC = TPC * E              # 40960 contraction columns per core
DD = KPC // 256            # 160 double-k-chunks (DoubleRow processes 256 k/step)
NH = 2                     # n-split halves of D1
NHD = D1 // NH             # 256 output cols per half
BPC = B // NCORES          # 32 batch rows per core after ReduceScatter

# stream pieces: small first piece so matmul 0 starts early
PIECES = [(0, 2)] + [(2 + 8 * i, 8) for i in range(19)] + [(154, 6)]

EMB_SCALE = 2.0 ** 13      # max|emb| 0.0109 -> 89  (fp8e4 normal range)
W1_SCALE = 2.0 ** 16       # max|W1|  0.00175 -> 115
DESCALE = 1.0 / (EMB_SCALE * W1_SCALE)

_prog_cache = {}
LAST_RESULTS = None        # BassKernelResults of the last kernel() call


def _install_ntff_hook():
    """Register the axon NTFF profile hook (image's antenv lacks axon_hooks)."""
    if "antenv.axon_hooks" in sys.modules:
        return
    mod = types.ModuleType("antenv.axon_hooks")
    mod._hook = None
    mod.set_axon_ntff_profile_hook = lambda h: setattr(mod, "_hook", h)
    mod.get_axon_ntff_profile_hook = lambda: mod._hook
    sys.modules["antenv.axon_hooks"] = mod
    import antenv

    antenv.axon_hooks = mod
    try:
        from trn_agent_boot.trn_boot import _ntff_profile_via_ctypes

        hook = _ntff_profile_via_ctypes("/opt/axon/libaxon_pjrt.so")
        if hook is not None:
            mod.set_axon_ntff_profile_hook(hook)
    except Exception:
        pass


def _build_program():
    if "nc" in _prog_cache:
        return _prog_cache["nc"]

    nc = bacc.Bacc("TRN2", num_devices=NCORES)
    f8, f16, f32 = mybir.dt.float8e4, mybir.dt.float16, mybir.dt.float32
    Relu = mybir.ActivationFunctionType.Relu
    Copy = mybir.ActivationFunctionType.Copy
    DR = mybir.MatmulPerfMode.DoubleRow

    x8 = nc.declare_dram_parameter("x8", [128, DD, 2, B], f8, isOutput=False)
    w1q = nc.declare_dram_parameter("w1q", [NH, 128, DD, 2, NHD], f8, isOutput=False)
    b1t = nc.declare_dram_parameter("b1t", [128, D1 // 128], f32, isOutput=False)
    w2t = nc.declare_dram_parameter("w2t", [D1, D2], f16, isOutput=False)
    b2c = nc.declare_dram_parameter("b2c", [128, D2 // 128], f32, isOutput=False)
    wpt = nc.declare_dram_parameter("wpt", [D2, C], f16, isOutput=False)
    bpc = nc.declare_dram_parameter("bpc", [C, 1], f32, isOutput=False)
    out = nc.declare_dram_parameter("out", [C, BPC], f32, isOutput=True)

    partial = [nc.dram_tensor(f"partial{h}", [B, NHD], f16) for h in range(NH)]
    y1scat = [nc.dram_tensor(f"y1scat{h}", [BPC, NHD], f16) for h in range(NH)]
    warm_in = nc.dram_tensor("warm_in", [2, 1], f32)
    warm_out = nc.dram_tensor("warm_out", [2, 1], f32, addr_space="Shared")

    with tile.TileContext(nc) as tc:
        with (
            tc.tile_pool(name="cpool", bufs=1) as cpool,
            tc.tile_pool(name="wpool", bufs=3) as wpool,
            tc.tile_pool(name="psum", bufs=1, space="PSUM") as pp,
        ):
            identity = cpool.tile([128, 128], f16)
            make_identity(nc, identity[:, :])

            ps1 = [
                [
                    pp.tile([128, NHD], f32, tag=f"ps1_{h}_{bc}", name=f"ps1_{h}_{bc}")
                    for bc in range(2)
                ]
                for h in range(NH)
            ]
            y1p = [
                cpool.tile([128, 2, NHD], f16, tag=f"y1p{h}", name=f"y1p{h}")
                for h in range(NH)
            ]

            # ---- layer-1 phase 0: x + W half 0, just-in-time pairs on the
            # sync ring (single FIFO = deterministic demand-order pacing) ----
            x_sb = cpool.tile([128, DD, 2, B], f8)
            for g, (d0, ln) in enumerate(PIECES):
                nc.sync.dma_start(
                    out=x_sb[:, d0 : d0 + ln, :, :], in_=x8[:, d0 : d0 + ln, :, :]
                )
                wb = wpool.tile([128, 8, 2, NHD], f8, tag="w1")
                nc.sync.dma_start(
                    out=wb[:, 0:ln, :, :], in_=w1q[0, :, d0 : d0 + ln, :, :]
                )
                for kk in range(ln):
                    dd = d0 + kk
                    for bc in range(2):
                        nc.tensor.matmul(
                            ps1[0][bc][:, :],
                            x_sb[:, dd, :, bc * 128 : (bc + 1) * 128],
                            wb[:, kk, :, :],
                            start=(dd == 0),
                            stop=(dd == DD - 1),
                            perf_mode=DR,
                        )

            # phase-0 readout + first ReduceScatter (overlaps phase 1)
            for bc in range(2):
                nc.scalar.activation(
                    out=y1p[0][:, bc, :], in_=ps1[0][bc][:, :], func=Copy,
                    scale=DESCALE,
                )
                nc.scalar.dma_start(
                    out=partial[0][bc * 128 : (bc + 1) * 128, :], in_=y1p[0][:, bc, :]
                )
            nc.gpsimd.collective_compute(
                "ReduceScatter",
                mybir.AluOpType.add,
                replica_groups=[list(range(NCORES))],
                ins=[partial[0][:, :]],
                outs=[y1scat[0][:, :]],
            )

            # ---- layer-1 phase 1: W half 1 only (PE-bound, DMA slack) ----
            for g, (d0, ln) in enumerate(PIECES):
                wb = wpool.tile([128, 8, 2, NHD], f8, tag="w1")
                nc.sync.dma_start(
                    out=wb[:, 0:ln, :, :], in_=w1q[1, :, d0 : d0 + ln, :, :]
                )
                for kk in range(ln):
                    dd = d0 + kk
                    for bc in range(2):
                        nc.tensor.matmul(
                            ps1[1][bc][:, :],
                            x_sb[:, dd, :, bc * 128 : (bc + 1) * 128],
                            wb[:, kk, :, :],
                            start=(dd == 0),
                            stop=(dd == DD - 1),
                            perf_mode=DR,
                        )

            # small tail constants (scalar queue, behind the x pieces)
            b1_sb = cpool.tile([128, D1 // 128], f32)
            nc.scalar.dma_start(out=b1_sb[:, :], in_=b1t[:, :])
            b2_sb = cpool.tile([128, D2 // 128], f32)
            nc.scalar.dma_start(out=b2_sb[:, :], in_=b2c[:, :])
            bp_sb = cpool.tile([C, 1], f32)
            nc.scalar.dma_start(out=bp_sb[:, :], in_=bpc[:, :])
            w2t_sb = cpool.tile([128, D1 // 128, D2], f16)
            nc.scalar.dma_start(
                out=w2t_sb[:, :, :], in_=w2t[:, :].rearrange("(c p) n -> p c n", p=128)
            )
            wpt_sb = cpool.tile([128, D2 // 128, C], f16)
            nc.scalar.dma_start(
                out=wpt_sb[:, :, :], in_=wpt[:, :].rearrange("(c p) n -> p c n", p=128)
            )

            # phase-1 readout + second ReduceScatter
            for bc in range(2):
                nc.scalar.activation(
                    out=y1p[1][:, bc, :], in_=ps1[1][bc][:, :], func=Copy,
                    scale=DESCALE,
                )
                nc.scalar.dma_start(
                    out=partial[1][bc * 128 : (bc + 1) * 128, :], in_=y1p[1][:, bc, :]
                )
            nc.gpsimd.collective_compute(
                "ReduceScatter",
                mybir.AluOpType.add,
                replica_groups=[list(range(NCORES))],
                ins=[partial[1][:, :]],
                outs=[y1scat[1][:, :]],
            )

            # ---- tail: this core's 32-row slice through layers 2/3 ----
            # tile_wait_until keeps the scheduler from hoisting tail PE work
            # (which waits on the collectives) into the layer-1 stream
            tail_ctx = tc.tile_wait_until(0.5)
            tail_ctx.__enter__()
            x1T = cpool.tile([128, D1 // 128, BPC], f16)
            psT = pp.tile([128, D1 // 128, BPC], f16, tag="pstr", name="psT")
            ps2 = pp.tile([128, D2 // 128, BPC], f32, tag="ps2", name="ps2")
            x1h = [
                cpool.tile([BPC, NHD], f16, tag=f"x1h{h}", name=f"x1h{h}")
                for h in range(NH)
            ]
            for h in range(NH):
                nc.scalar.dma_start(out=x1h[h][:, :], in_=y1scat[h][:, :])
                for cc2 in range(NHD // 128):
                    cc = h * (NHD // 128) + cc2
                    nc.tensor.transpose(
                        psT[:, cc, :],
                        x1h[h][:, cc2 * 128 : (cc2 + 1) * 128],
                        identity[0:BPC, 0:BPC],
                    )
                    nc.scalar.activation(
                        out=x1T[:, cc, :],
                        in_=psT[:, cc, :],
                        func=Relu,
                        bias=b1_sb[:, cc : cc + 1],
                        scale=1.0,
                    )
            for mc in range(D2 // 128):
                for kc in range(D1 // 128):
                    nc.tensor.matmul(
                        ps2[:, mc, :],
                        w2t_sb[:, kc, mc * 128 : (mc + 1) * 128],
                        x1T[:, kc, :],
                        start=(kc == 0),
                        stop=(kc == D1 // 128 - 1),
                    )
            x2T = cpool.tile([128, D2 // 128, BPC], f16)
            for mc in range(D2 // 128):
                nc.scalar.activation(
                    out=x2T[:, mc, :],
                    in_=ps2[:, mc, :],
                    func=Relu,
                    bias=b2_sb[:, mc : mc + 1],
                    scale=1.0,
                )

            ps3 = pp.tile([C, BPC], f32, tag="ps3")
            for kc in range(D2 // 128):
                nc.tensor.matmul(
                    ps3[:, :],
                    wpt_sb[:, kc, :],
                    x2T[:, kc, :],
                    start=(kc == 0),
                    stop=(kc == D2 // 128 - 1),
                )
            logits = cpool.tile([C, BPC], f32)
            nc.vector.tensor_scalar_add(logits[:, :], ps3[:, :], bp_sb[:, 0:1])
            nc.sync.dma_start(out=out[:, :], in_=logits[:, :])
            tail_ctx.__exit__(None, None, None)

    nc.finalize()
    _prog_cache["nc"] = nc
    return nc


def _host_prep(data, mask, emb_table, W1, b1, W2, b2, Wp, bp):
    f8 = mybir.dt.np(mybir.dt.float8e4)
    data = np.asarray(data)
    mask = np.asarray(mask)
    tokens = np.where(mask != 0, data, V).astype(np.int64)  # V -> zero row
    emb8 = np.vstack(
        [
            (np.asarray(emb_table) * EMB_SCALE).astype(f8),
            np.zeros((1, E), f8),
        ]
    )
    W1 = np.asarray(W1)
    b1_in = np.asarray(b1).astype(np.float32).reshape(D1 // 128, 128).T.copy()
    W2T = np.ascontiguousarray(np.asarray(W2).astype(np.float16).T)
    b2_in = np.asarray(b2).astype(np.float32).reshape(D2 // 128, 128).T.copy()
    WpT = np.ascontiguousarray(np.asarray(Wp).astype(np.float16).T)
    bp_in = np.asarray(bp).astype(np.float32).reshape(C, 1)

    in_maps = []
    for c in range(NCORES):
        toks_c = tokens[:, c * TPC : (c + 1) * TPC]          # [B, TPC]
        xg = emb8[toks_c]                                    # [B, TPC, E] fp8
        # k-major: k = t*E + e -> [dd, pair, p] ; lhsT layout [p, dd, pair, b]
        x8c = np.ascontiguousarray(
            xg.reshape(B, DD, 2, 128).transpose(3, 1, 2, 0)
        )
        w1c = (W1[:, c * KPC : (c + 1) * KPC] * W1_SCALE).astype(f8)  # [512, 40960]
        # [n, dd, pair, p] -> [p, dd, pair, n] -> split n halves
        w1k = w1c.reshape(D1, DD, 2, 128).transpose(3, 1, 2, 0)
        w1q_c = np.ascontiguousarray(
            np.stack([w1k[..., h * NHD : (h + 1) * NHD] for h in range(NH)])
        )
        in_maps.append(
            {
                "x8": x8c,
                "w1q": w1q_c,
                "b1t": b1_in,
                "w2t": W2T,
                "b2c": b2_in,
                "wpt": WpT,
                "bpc": bp_in,
            }
        )
    return in_maps


def kernel(data, mask, emb_table, W1, b1, W2, b2, Wp, bp):
    global LAST_RESULTS
    nc = _build_program()
    in_maps = _host_prep(data, mask, emb_table, W1, b1, W2, b2, Wp, bp)

    trace = os.environ.get("KERNEL_TRACE", "0") == "1"
    if trace:
        _install_ntff_hook()
    br = run_bass_kernel_spmd(nc, in_maps, list(range(NCORES)), trace=trace)
    LAST_RESULTS = br
    full = np.concatenate(
        [np.asarray(br.results[c]["out"]) for c in range(NCORES)], axis=1
    )
    return np.ascontiguousarray(full.T.astype(np.float32))


# revision 22
# speedup vs baseline: 1.1339x; 1.1339x over previous
"""Trainium2 Bass kernel for nn_AttnTextClassifier (fp8 DoubleRow version).

Reference math (B=256, T=512, V=50000, E=640, D1=D2=512, C=2):
    tokens   = data * mask                     [B, T]
    embedded = emb_table[tokens] * mask[...,None]
    x  = embedded.reshape(B, T*E)              [B, 327680]
    x1 = relu(x @ W1.T + b1)                   [B, 512]
    x2 = relu(x1 @ W2.T + b2)                  [B, 512]
    out = x2 @ Wp.T + bp                       [B, 2]

Distribution (8 cores): tensor-parallel over the T*E contraction dim.
Core c owns tokens t in [64c, 64c+64) -> 40960 contraction columns.

Both emb_table and W1 are uniform-init, so fp8e4 (scaled by powers of two
into the normal range) keeps the end-to-end max rel err ~1.5e-3 (measured
host-side), well inside the 2e-2 gate.  The host pre-gathers the embedding
rows into k-major fp8 tiles (no on-device gather, no on-device transposes)
and pre-transposes W1; the device runs a pure fp8 DoubleRow matmul stream
(2 fp8 weights/PE cell, warm matmul issue-to-issue 132 ns).  x and W
stream just-in-time in matched pieces, which sustains ~330 GB/s.

The D1=512 output columns are split in two phases.  Phase 0 moves x + the
first W half (DMA-bound); phase 1 moves only the second W half and is
PE-bound, leaving DMA slack in which the ncfw collective path (booted by
a t~8us warm-up AllReduce) executes the phase-0 ReduceScatter.  Only the
phase-1 ReduceScatter is exposed at the end.  Each core then computes
layers 2/3 for its own 32-row batch slice (layer-2 partials for the
n-half-0 columns are computed while ReduceScatter #2 is in flight) and
the host concatenates the 8 output slices (pure unshard).
"""

import os
import sys
import types

import numpy as np

import concourse.bacc as bacc
import concourse.mybir as mybir
import concourse.tile as tile
from concourse.bass_utils import run_bass_kernel_spmd
from concourse.masks import make_identity

B, T, V, E = 256, 512, 50000, 640
D1, D2, C = 512, 512, 2
NCORES = 8
TPC = T // NCORES          # 64 tokens per core
KPC = TPC * E              # 40960 contraction columns per core
DD = KPC // 256            # 160 double-k-chunks (DoubleRow processes 256 k/step)
NH = 2                     # n-split halves of D1
NHD = D1 // NH             # 256 output cols per half
BPC = B // NCORES          # 32 batch rows per core after ReduceScatter

# stream pieces: small first piece so matmul 0 starts early
PIECES = [(0, 2)] + [(2 + 8 * i, 8) for i in range(19)] + [(154, 6)]

EMB_SCALE = 2.0 ** 13      # max|emb| 0.0109 -> 89  (fp8e4 normal range)
W1_SCALE = 2.0 ** 16       # max|W1|  0.00175 -> 115
DESCALE = 1.0 / (EMB_SCALE * W1_SCALE)

_prog_cache = {}
LAST_RESULTS = None        # BassKernelResults of the last kernel() call


def _install_ntff_hook():
    """Register the axon NTFF profile hook (image's antenv lacks axon_hooks)."""
    if "antenv.axon_hooks" in sys.modules:
        return
    mod = types.ModuleType("antenv.axon_hooks")
    mod._hook = None
    mod.set_axon_ntff_profile_hook = lambda h: setattr(mod, "_hook", h)
    mod.get_axon_ntff_profile_hook = lambda: mod._hook
    sys.modules["antenv.axon_hooks"] = mod
    import antenv

    antenv.axon_hooks = mod
    try:
        from trn_agent_boot.trn_boot import _ntff_profile_via_ctypes

        hook = _ntff_profile_via_ctypes("/opt/axon/libaxon_pjrt.so")
        if hook is not None:
            mod.set_axon_ntff_profile_hook(hook)
    except Exception:
        pass


def _build_program():
    if "nc" in _prog_cache:
        return _prog_cache["nc"]

    nc = bacc.Bacc("TRN2", num_devices=NCORES)
    f8, f16, f32 = mybir.dt.float8e4, mybir.dt.float16, mybir.dt.float32
    Relu = mybir.ActivationFunctionType.Relu
    Copy = mybir.ActivationFunctionType.Copy
    DR = mybir.MatmulPerfMode.DoubleRow

    x8 = nc.declare_dram_parameter("x8", [128, DD, 2, B], f8, isOutput=False)
    w1q = nc.declare_dram_parameter("w1q", [NH, 128, DD, 2, NHD], f8, isOutput=False)
    b1t = nc.declare_dram_parameter("b1t", [128, D1 // 128], f32, isOutput=False)
    w2t = nc.declare_dram_parameter("w2t", [D1, D2], f16, isOutput=False)
    b2c = nc.declare_dram_parameter("b2c", [128, D2 // 128], f32, isOutput=False)
    wpt = nc.declare_dram_parameter("wpt", [D2, C], f16, isOutput=False)
    bpc = nc.declare_dram_parameter("bpc", [C, 1], f32, isOutput=False)
    out = nc.declare_dram_parameter("out", [C, BPC], f32, isOutput=True)

    partial = [nc.dram_tensor(f"partial{h}", [B, NHD], f16) for h in range(NH)]
    y1scat = [nc.dram_tensor(f"y1scat{h}", [BPC, NHD], f16) for h in range(NH)]
    warm_in = nc.dram_tensor("warm_in", [2, 1], f32)
    warm_out = nc.dram_tensor("warm_out", [2, 1], f32, addr_space="Shared")

    with tile.TileContext(nc) as tc:
        with (
            tc.tile_pool(name="cpool", bufs=1) as cpool,
            tc.tile_pool(name="wpool", bufs=5) as wpool,
            tc.tile_pool(name="psum", bufs=1, space="PSUM") as pp,
        ):
            # warm up the ncfw collective path (boots while layer 1 streams)
            nc.sync.dma_start(out=warm_in[:, :], in_=bpc[:, :])
            nc.gpsimd.collective_compute(
                "AllReduce",
                mybir.AluOpType.add,
                replica_groups=[list(range(NCORES))],
                ins=[warm_in[:, :]],
                outs=[warm_out[:, :]],
            )

            identity = cpool.tile([128, 128], f16)
            make_identity(nc, identity[:, :])

            ps1 = [
                [
                    pp.tile([128, NHD], f32, tag=f"ps1_{h}_{bc}", name=f"ps1_{h}_{bc}")
                    for bc in range(2)
                ]
                for h in range(NH)
            ]
            y1p = [
                cpool.tile([128, 2, NHD], f16, tag=f"y1p{h}", name=f"y1p{h}")
                for h in range(NH)
            ]

            # ---- layer-1 phase 0: x + W half 0, just-in-time pairs on the
            # sync ring (single FIFO = deterministic demand-order pacing) ----
            x_sb = cpool.tile([128, DD, 2, B], f8)
            for g, (d0, ln) in enumerate(PIECES):
                nc.sync.dma_start(
                    out=x_sb[:, d0 : d0 + ln, :, :], in_=x8[:, d0 : d0 + ln, :, :]
                )
                wb = wpool.tile([128, 8, 2, NHD], f8, tag="w1")
                nc.sync.dma_start(
                    out=wb[:, 0:ln, :, :], in_=w1q[0, :, d0 : d0 + ln, :, :]
                )
                for kk in range(ln):
                    dd = d0 + kk
                    for bc in range(2):
                        nc.tensor.matmul(
                            ps1[0][bc][:, :],
                            x_sb[:, dd, :, bc * 128 : (bc + 1) * 128],
                            wb[:, kk, :, :],
                            start=(dd == 0),
                            stop=(dd == DD - 1),
                            perf_mode=DR,
                        )

            # phase-0 readout + first ReduceScatter (overlaps phase 1)
            for bc in range(2):
                nc.scalar.activation(
                    out=y1p[0][:, bc, :], in_=ps1[0][bc][:, :], func=Copy,
                    scale=DESCALE,
                )
                nc.scalar.dma_start(
                    out=partial[0][bc * 128 : (bc + 1) * 128, :], in_=y1p[0][:, bc, :]
                )
            nc.gpsimd.collective_compute(
                "ReduceScatter",
                mybir.AluOpType.add,
                replica_groups=[list(range(NCORES))],
                ins=[partial[0][:, :]],
                outs=[y1scat[0][:, :]],
            )

            # ---- layer-1 phase 1: W half 1 only (PE-bound, DMA slack) ----
            for g, (d0, ln) in enumerate(PIECES):
                wb = wpool.tile([128, 8, 2, NHD], f8, tag="w1")
                nc.sync.dma_start(
                    out=wb[:, 0:ln, :, :], in_=w1q[1, :, d0 : d0 + ln, :, :]
                )
                for kk in range(ln):
                    dd = d0 + kk
                    for bc in range(2):
                        nc.tensor.matmul(
                            ps1[1][bc][:, :],
                            x_sb[:, dd, :, bc * 128 : (bc + 1) * 128],
                            wb[:, kk, :, :],
                            start=(dd == 0),
                            stop=(dd == DD - 1),
                            perf_mode=DR,
                        )

            # small tail constants (scalar queue, behind the x pieces)
            b1_sb = cpool.tile([128, D1 // 128], f32)
            nc.scalar.dma_start(out=b1_sb[:, :], in_=b1t[:, :])
            b2_sb = cpool.tile([128, D2 // 128], f32)
            nc.scalar.dma_start(out=b2_sb[:, :], in_=b2c[:, :])
            bp_sb = cpool.tile([C, 1], f32)
            nc.scalar.dma_start(out=bp_sb[:, :], in_=bpc[:, :])
            w2t_sb = cpool.tile([128, D1 // 128, D2], f16)
            nc.scalar.dma_start(
                out=w2t_sb[:, :, :], in_=w2t[:, :].rearrange("(c p) n -> p c n", p=128)
            )
            wpt_sb = cpool.tile([128, D2 // 128, C], f16)
            nc.scalar.dma_start(
                out=wpt_sb[:, :, :], in_=wpt[:, :].rearrange("(c p) n -> p c n", p=128)
            )

            # phase-1 readout + second ReduceScatter
            for bc in range(2):
                nc.scalar.activation(
                    out=y1p[1][:, bc, :], in_=ps1[1][bc][:, :], func=Copy,
                    scale=DESCALE,
                )
                nc.scalar.dma_start(
                    out=partial[1][bc * 128 : (bc + 1) * 128, :], in_=y1p[1][:, bc, :]
                )
            nc.gpsimd.collective_compute(
                "ReduceScatter",
                mybir.AluOpType.add,
                replica_groups=[list(range(NCORES))],
                ins=[partial[1][:, :]],
                outs=[y1scat[1][:, :]],
            )

            # ---- tail: this core's 32-row slice through layers 2/3 ----
            # tile_wait_until keeps the scheduler from hoisting tail PE work
            # (which waits on the collectives) into the layer-1 stream
            tail_ctx = tc.tile_wait_until(0.5)
            tail_ctx.__enter__()
            x1T = cpool.tile([128, D1 // 128, BPC], f16)
            psT = pp.tile([128, D1 // 128, BPC], f16, tag="pstr", name="psT")
            ps2 = pp.tile([128, D2 // 128, BPC], f32, tag="ps2", name="ps2")
            x1h = [
                cpool.tile([BPC, NHD], f16, tag=f"x1h{h}", name=f"x1h{h}")
                for h in range(NH)
            ]
            for h in range(NH):
                nc.scalar.dma_start(out=x1h[h][:, :], in_=y1scat[h][:, :])
                for cc2 in range(NHD // 128):
                    cc = h * (NHD // 128) + cc2
                    nc.tensor.transpose(
                        psT[:, cc, :],
                        x1h[h][:, cc2 * 128 : (cc2 + 1) * 128],
                        identity[0:BPC, 0:BPC],
                    )
                    nc.scalar.activation(
                        out=x1T[:, cc, :],
                        in_=psT[:, cc, :],
                        func=Relu,
                        bias=b1_sb[:, cc : cc + 1],
                        scale=1.0,
                    )
            for mc in range(D2 // 128):
                for kc in range(D1 // 128):
                    nc.tensor.matmul(
                        ps2[:, mc, :],
                        w2t_sb[:, kc, mc * 128 : (mc + 1) * 128],
                        x1T[:, kc, :],
                        start=(kc == 0),
                        stop=(kc == D1 // 128 - 1),
                    )
            x2T = cpool.tile([128, D2 // 128, BPC], f16)
            for mc in range(D2 // 128):
                nc.scalar.activation(
                    out=x2T[:, mc, :],
                    in_=ps2[:, mc, :],
                    func=Relu,
                    bias=b2_sb[:, mc : mc + 1],
                    scale=1.0,
                )

            ps3 = pp.tile([C, BPC], f32, tag="ps3")
            for kc in range(D2 // 128):
                nc.tensor.matmul(
                    ps3[:, :],
                    wpt_sb[:, kc, :],
                    x2T[:, kc, :],
                    start=(kc == 0),
                    stop=(kc == D2 // 128 - 1),
                )
            logits = cpool.tile([C, BPC], f32)
            nc.vector.tensor_scalar_add(logits[:, :], ps3[:, :], bp_sb[:, 0:1])
            nc.sync.dma_start(out=out[:, :], in_=logits[:, :])
            tail_ctx.__exit__(None, None, None)

    nc.finalize()
    _prog_cache["nc"] = nc
    return nc


def _host_prep(data, mask, emb_table, W1, b1, W2, b2, Wp, bp):
    f8 = mybir.dt.np(mybir.dt.float8e4)
    data = np.asarray(data)
    mask = np.asarray(mask)
    tokens = np.where(mask != 0, data, V).astype(np.int64)  # V -> zero row
    emb8 = np.vstack(
        [
            (np.asarray(emb_table) * EMB_SCALE).astype(f8),
            np.zeros((1, E), f8),
        ]
    )
    W1 = np.asarray(W1)
    b1_in = np.asarray(b1).astype(np.float32).reshape(D1 // 128, 128).T.copy()
    W2T = np.ascontiguousarray(np.asarray(W2).astype(np.float16).T)
    b2_in = np.asarray(b2).astype(np.float32).reshape(D2 // 128, 128).T.copy()
    WpT = np.ascontiguousarray(np.asarray(Wp).astype(np.float16).T)
    bp_in = np.asarray(bp).astype(np.float32).reshape(C, 1)

    in_maps = []
    for c in range(NCORES):
        toks_c = tokens[:, c * TPC : (c + 1) * TPC]          # [B, TPC]
        xg = emb8[toks_c]                                    # [B, TPC, E] fp8
        # k-major: k = t*E + e -> [dd, pair, p] ; lhsT layout [p, dd, pair, b]
        x8c = np.ascontiguousarray(
            xg.reshape(B, DD, 2, 128).transpose(3, 1, 2, 0)
        )
        w1c = (W1[:, c * KPC : (c + 1) * KPC] * W1_SCALE).astype(f8)  # [512, 40960]
        # [n, dd, pair, p] -> [p, dd, pair, n] -> split n halves
        w1k = w1c.reshape(D1, DD, 2, 128).transpose(3, 1, 2, 0)
        w1q_c = np.ascontiguousarray(
            np.stack([w1k[..., h * NHD : (h + 1) * NHD] for h in range(NH)])
        )
        in_maps.append(
            {
                "x8": x8c,
                "w1q": w1q_c,
                "b1t": b1_in,
                "w2t": W2T,
                "b2c": b2_in,
                "wpt": WpT,
                "bpc": bp_in,
            }
        )
    return in_maps


def kernel(data, mask, emb_table, W1, b1, W2, b2, Wp, bp):
    global LAST_RESULTS
    nc = _build_program()
    in_maps = _host_prep(data, mask, emb_table, W1, b1, W2, b2, Wp, bp)

    trace = os.environ.get("KERNEL_TRACE", "0") == "1"
    if trace:
        _install_ntff_hook()
    br = run_bass_kernel_spmd(nc, in_maps, list(range(NCORES)), trace=trace)
    LAST_RESULTS = br
    full = np.concatenate(
        [np.asarray(br.results[c]["out"]) for c in range(NCORES)], axis=1
    )
    return np.ascontiguousarray(full.T.astype(np.float32))
